# revision 22
# baseline (speedup 1.0000x reference)
import sys as _sys
for _p in ('/opt/trn_rl_repo',):
    if _p not in _sys.path:
        _sys.path.insert(0, _p)
"""AnalyticBlock Trainium kernel: channel-sharded SPMD across 8 cores.

Per core: S = 34 channel slabs (1 halo each side), B batch, 128x128
images. Layout per slab: SBUF [h=128 partitions, (b, w_pad=132)], fp16.

Software-pipelined slot schedule (slot c, PE quarters q0..q3):
  PE  : 8 banded matmuls/quarter -> gx,gy,lap in PSUM (2D conv via
        vertical stationaries x horizontally shifted moving APs);
        T3(c-7) s,t broadcast @q1; T2(c-5) stats partition-reduce @q3
  ACT : sumsq(c-5) = Square(mixed)+accum_out at slot start (fills the
        PE-wait boundary gap); per quarter Square (psum->gxy2 fp16,
        quarter-contiguous) + Abs (psum lap -> lapa); A3(c-6) std
        sqrt @q1; full-slab Sqrt(c-1) @q2
  DVE : q(c-1) = gx2+gy2 (tt); feat(c-2) = gms+lapa (tt); A4(c-7)
        bcast copy; mix(c-4): t1,t2 (ts 4x), a (tt 2x), mixed (stt
        + sum accum_out); pass2(c-7): inner (ts), scr2 (ts),
        out = scr2+inner (tt, fp16 out); BN scalar chains (c-6)
        interleaved between big ops so s_ty write-acks never stall
  Chains: negvar = mu^2-E2 (stt); std = Sqrt(-negvar+eps) on ACT;
        s = 1/std * gamma*bn_w; tneg = s*mu - gamma*bn_b;
        inner = (1-gamma)*x - tneg.
  BN is per-channel over (B,H,W): fully local under channel sharding,
  no collectives. Output written fp16 (halves DMA), upcast on host.
"""
import math
import numpy as np
import ml_dtypes

import concourse.bass as bass
import concourse.mybir as mybir

F32 = mybir.dt.float32
FP16 = mybir.dt.float16
ALU = mybir.AluOpType
ACTF = mybir.ActivationFunctionType

BN_EPS = 1e-5
MIX_EPS = 1e-3
H = 128
W = 128
WP = 132  # 2 zero pad cols each side


def _softplus(x):
    return np.logaddexp(0.0, x)


def conv_stationaries():
    """6 stationaries [K=128, M=128], lhsT layout: lhsT[k, m] = M[m, k]."""
    n = H
    i = np.arange(n)
    Vs = np.zeros((n, n), np.float32)   # vertical smooth [1,2,1]
    Vs[i, i] = 2.0
    Vs[i[1:], i[1:] - 1] = 1.0
    Vs[i[:-1], i[:-1] + 1] = 1.0
    Vd = np.zeros((n, n), np.float32)   # vertical corr [1,0,-1]
    Vd[i[1:], i[1:] - 1] = 1.0
    Vd[i[:-1], i[:-1] + 1] = -1.0
    VL = np.zeros((n, n), np.float32)   # vertical lap incl center
    VL[i, i] = -4.0
    VL[i[1:], i[1:] - 1] = 1.0
    VL[i[:-1], i[:-1] + 1] = 1.0
    I = np.eye(n, dtype=np.float32)
    mats = [Vs, -Vs, Vd, 2.0 * Vd, VL, I]
    cw = np.stack([m.T for m in mats], axis=1)  # [K=128, 6, M=128]
    return cw.astype(np.float16)


def build_nc(S, B, NQ, scal, n_cores=8, debug=False):
    NO = S - 2
    BQ = B // NQ
    QC = BQ * W            # cols per quarter
    COLS = B * W           # cols per slab
    NTOT = B * H * W
    a2 = scal["a_gm"] ** 2
    a_lap = scal["a_lap"]
    one_mg = 1.0 - scal["gamma"]

    XR, FR, MR, LR = 11, 4, 4, 3
    NC = S + 7             # slots: DMA(o=c-8) runs to c = NO+8 = S+6

    nc = bass.Bass(target_bir_lowering=False, detect_race_conditions=False)

    xb_ext = nc.declare_dram_parameter("xb", [S, H, B, WP], FP16, isOutput=False)
    cw_ext = nc.declare_dram_parameter("cw", [H, 6, H], FP16, isOutput=False)
    wmix_ext = nc.declare_dram_parameter("wmix", [H, 3 * NO], F32, isOutput=False)
    wbn_ext = nc.declare_dram_parameter("wbn", [1, 2 * NO], F32, isOutput=False)
    ones_ext = nc.declare_dram_parameter("onesv", [1, H], F32, isOutput=False)
    cb_ext = nc.declare_dram_parameter("cb", [H, 3], F32, isOutput=False)
    out_ext = nc.declare_dram_parameter("out", [NO, H, B, W], FP16, isOutput=True)

    from contextlib import ExitStack
    ctx = ExitStack()
    sb = lambda name, shape, dt: ctx.enter_context(nc.sbuf_tensor(name, shape, dt))
    ps = lambda name, shape: ctx.enter_context(nc.psum_tensor(name, shape, F32))
    sem = lambda name: ctx.enter_context(nc.semaphore(name))

    xb_t = [sb(f"xb{j}", [H, B * WP], FP16) for j in range(XR)]
    cw_t = sb("cw_s", [H, 6 * H], FP16)
    wmix_t = sb("wmix_s", [H, 3 * NO], F32)
    wbn_t = sb("wbn_s", [1, 2 * NO], F32)
    ones_bc = sb("ones_bc", [1, H], F32)
    cb_t = sb("cb_s", [H, 3], F32)   # col0 sqrt bias, col1 BN_EPS, col2 1.0

    feat_t = [sb(f"feat{j}", [H, COLS], FP16) for j in range(FR)]
    mixed_t = [sb(f"mixed{j}", [H, COLS], FP16) for j in range(MR)]
    gxy2_t = [sb(f"gxy2_{j}", [H, 2 * COLS], FP16) for j in range(2)]
    lapa_t = [sb(f"lapa{j}", [H, COLS], FP16) for j in range(LR)]
    qsl_t = [sb(f"qsl{j}", [H, COLS], FP16) for j in range(2)]
    gms_t = [sb(f"gms{j}", [H, COLS], FP16) for j in range(2)]
    t1_t = sb("t1", [H, COLS], FP16)
    t2_t = sb("t2", [H, COLS], FP16)
    a_t = sb("a_t", [H, COLS], FP16)
    gscr = sb("gscr", [H, COLS], FP16)
    inner_t = sb("inner", [H, COLS], FP16)
    scr2_t = sb("scr2", [H, COLS], FP16)
    out_sb = [sb(f"outsb{j}", [H, COLS], FP16) for j in range(2)]
    stats_t = [sb(f"stats{j}", [H, 2], F32) for j in range(2)]
    sc_mu = sb("sc_mu", [1, 2], F32)
    sc_tmp = sb("sc_tmp", [1, 1], F32)
    sc_var = [sb(f"sc_var{j}", [1, 1], F32) for j in range(2)]
    sc_std = [sb(f"sc_std{j}", [1, 1], F32) for j in range(2)]
    sc_inv = sb("sc_inv", [1, 1], F32)
    st_vec = [sb(f"st_vec{j}", [1, 2], F32) for j in range(2)]
    sb_st = [sb(f"sb_st{j}", [H, 2], F32) for j in range(2)]

    psum_gxy = [ps(f"psgxy{j}", [H, 2 * QC]) for j in range(2)]
    psum_lap = [ps(f"pslap{j}", [H, QC]) for j in range(2)]
    psum_stat = ps("psstat", [1, 6])
    psum_bc = ps("psbc", [H, 4])

    s_cst = sem("s_cst")
    s_xb = [sem(f"s_xb{j}") for j in range(XR)]
    s_mm = sem("s_mm")        # PE conv quarter done (count 4c+q+1)
    s_sq = sem("s_sq")        # ACT Square done (count 4c+q+1)
    s_psf = sem("s_psf")      # ACT Abs done -> psum slot free (count 4c+q+1)
    s_q = sem("s_q")          # GPS qadd(m) done (count m+1)
    s_sqrt = sem("s_sqrt")    # ACT Sqrt(m) done (count m+1)
    s_feat = sem("s_feat")    # GPS feat(m) done (count m+1)
    s_mixd = sem("s_mixd")    # DVE mixed(m) done (count m)
    s_ssq = sem("s_ssq")      # DVE sumsq(m) done (count m)
    s_statmm = sem("s_statmm")  # PE T2(o) done (count o)
    s_std = sem("s_std")      # ACT A3(o) done (count o)
    s_bc = sem("s_bc")        # PE T3(o) done (count o)
    s_stsb = sem("s_stsb")    # ACT A4(o) done (count o)
    s_v2 = sem("s_v2")        # DVE out(o) done (count o)
    s_ty = sem("s_ty")        # DVE tiny-op write-ack ordering
    s_do = [sem(f"s_do{j}") for j in range(2)]

    def xb_q(c, q, sh):
        v = xb_t[c % XR][:, :].rearrange("p (b w) -> p b w", w=WP)
        return v[:, q * BQ:(q + 1) * BQ, sh:sh + W]

    def xb_int(c):
        v = xb_t[c % XR][:, :].rearrange("p (b w) -> p b w", w=WP)
        return v[:, :, 2:2 + W]

    def cwm(j):
        return cw_t[:, j * H:(j + 1) * H]

    def gxy2_view(c, q):
        # quarter-contiguous [gx|gy] block: no strided ACT writes
        return gxy2_t[c % 2][:, q * 2 * QC:(q + 1) * 2 * QC]

    def gxy2_half(c, off):
        v = gxy2_t[c % 2][:, :].rearrange("p (qq x) -> p qq x", x=2 * QC)
        return v[:, :, off:off + QC]

    with nc.Block() as block:

        @block.sync
        def _(sync):
            sync.dma_start(out=cw_t[:, :], in_=cw_ext[:, :, :]).then_inc(s_cst, 16)
            sync.dma_start(out=wmix_t[:, :], in_=wmix_ext[:, :]).then_inc(s_cst, 16)
            sync.dma_start(out=wbn_t[:, :], in_=wbn_ext[:, :]).then_inc(s_cst, 16)
            sync.dma_start(out=ones_bc[0:1, :], in_=ones_ext[0:1, :]).then_inc(s_cst, 16)
            sync.dma_start(out=cb_t[:, :], in_=cb_ext[:, :]).then_inc(s_cst, 16)
            for c0 in range(min(3, S)):
                sync.dma_start(out=xb_t[c0 % XR][:, :],
                               in_=xb_ext[c0, :, :, :]).then_inc(s_xb[c0 % XR], 16)
            for c in range(NC):
                cl = c + 3
                if cl < S:
                    j = cl % XR
                    if cl >= XR:
                        sync.wait_ge(s_v2, cl - XR)
                    sync.dma_start(out=xb_t[j][:, :],
                                   in_=xb_ext[cl, :, :, :]).then_inc(s_xb[j], 16)
                o = c - 8
                if 1 <= o <= NO:
                    sync.wait_ge(s_v2, o)
                    sync.dma_start(out=out_ext[o - 1, :, :, :],
                                   in_=out_sb[o % 2][:, :]).then_inc(s_do[o % 2], 16)

        @block.tensor
        def _(tensor):
            tensor.wait_ge(s_cst, 80)
            for c in range(NC):
                for q in range(NQ):
                    if q == 1:
                        # T3(o = c-7): broadcast s,t -> [H,2] psum
                        o3 = c - 7
                        if 1 <= o3 <= NO:
                            tensor.wait_ge(s_ty, 5 * o3)
                            if o3 >= 3:
                                tensor.wait_ge(s_stsb, o3 - 2)
                            tensor.matmul(
                                psum_bc[:, (o3 % 2) * 2:(o3 % 2) * 2 + 2],
                                ones_bc[0:1, :], st_vec[o3 % 2][0:1, :],
                                start=True, stop=True).then_inc(s_bc, 1)
                    if c <= S - 1:
                        it = c * NQ + q
                        if q == 0:
                            tensor.wait_ge(s_xb[c % XR], (c // XR + 1) * 16)
                        if it >= 2:
                            tensor.wait_ge(s_psf, it - 1)
                        slot = it % 2
                        g = psum_gxy[slot]
                        l = psum_lap[slot]
                        mm = tensor.matmul
                        mm(g[:, 0:QC], cwm(0), xb_q(c, q, 1), start=True, stop=False)
                        mm(g[:, 0:QC], cwm(1), xb_q(c, q, 3), start=False, stop=True)
                        mm(g[:, QC:2 * QC], cwm(2), xb_q(c, q, 1), start=True, stop=False)
                        mm(g[:, QC:2 * QC], cwm(2), xb_q(c, q, 3), start=False, stop=False)
                        mm(g[:, QC:2 * QC], cwm(3), xb_q(c, q, 2), start=False, stop=True)
                        mm(l[:, :], cwm(5), xb_q(c, q, 1), start=True, stop=False)
                        mm(l[:, :], cwm(5), xb_q(c, q, 3), start=False, stop=False)
                        mm(l[:, :], cwm(4), xb_q(c, q, 2), start=False, stop=True) \
                            .then_inc(s_mm, 1)
                        # p-state keep-alive: 1-col junk matmul, no sems
                        mm(psum_stat[0:1, 4:5], cb_t[:, 2:3], cb_t[:, 0:1],
                           start=True, stop=True, skip_group_check=True)
                # T2(o = c-4): stats partition-reduce
                o2 = c - 5
                if 1 <= o2 <= NO:
                    tensor.wait_ge(s_ssq, o2)
                    if o2 >= 3:
                        tensor.wait_ge(s_ty, 5 * (o2 - 3) + 1)
                    tensor.matmul(psum_stat[0:1, (o2 % 2) * 2:(o2 % 2) * 2 + 2],
                                  cb_t[:, 2:3], stats_t[o2 % 2][:, :],
                                  start=True, stop=True).then_inc(s_statmm, 1)

        @block.scalar
        def _(scalar):
            act = scalar.activation
            scalar.wait_ge(s_cst, 80)
            for c in range(NC):
                # sumsq(m2 = c-5): PE-independent; fills slot-boundary gap
                m2 = c - 5
                if 1 <= m2 <= NO:
                    scalar.wait_ge(s_mixd, m2)
                    if m2 >= 3:
                        scalar.wait_ge(s_statmm, m2 - 2)
                    act(gscr[:, :], mixed_t[m2 % MR][:, :],
                        ACTF.Square,
                        accum_out=stats_t[m2 % 2][:, 1:2]).then_inc(s_ssq, 1)
                for q in range(NQ):
                    if c <= S - 1:
                        it = c * NQ + q
                        slot = it % 2
                        scalar.wait_ge(s_mm, it + 1)
                        if q == 0 and c >= 2:
                            scalar.wait_ge(s_q, c - 1)      # gxy2 slot free
                        act(gxy2_view(c, q), psum_gxy[slot][:, :],
                            ACTF.Square).then_inc(s_sq, 1)
                        if q == 0 and c >= 3:
                            scalar.wait_ge(s_feat, c - 2)   # lapa slot free
                        act(lapa_t[c % LR][:, q * QC:(q + 1) * QC],
                            psum_lap[slot][:, :], ACTF.Abs,
                            scale=a_lap).then_inc(s_psf, 1)
                    if q == 1:
                        # A3(o = c-6): std = sqrt(-negvar + eps)
                        o = c - 6
                        if 1 <= o <= NO:
                            scalar.wait_ge(s_ty, 5 * (o - 1) + 2)
                            if o >= 3:
                                scalar.wait_ge(s_ty, 5 * (o - 3) + 3)
                            act(sc_std[o % 2][0:1, :], sc_var[o % 2][0:1, :],
                                ACTF.Sqrt, bias=cb_t[0:1, 1:2],
                                scale=-1.0).then_inc(s_std, 1)
                    if q == 2:
                        # full-slab Sqrt(m = c-1)
                        m = c - 1
                        if 0 <= m <= S - 1:
                            scalar.wait_ge(s_q, m + 1)
                            if m >= 2:
                                scalar.wait_ge(s_feat, m - 1)  # gms slot free
                            act(gms_t[m % 2][:, :], qsl_t[m % 2][:, :],
                                ACTF.Sqrt, bias=cb_t[:, 0:1],
                                scale=a2).then_inc(s_sqrt, 1)

        @block.vector
        def _(vector):
            vector.wait_ge(s_cst, 80)
            stt = vector.scalar_tensor_tensor
            ts = vector.tensor_scalar
            tt = vector.tensor_tensor

            tyc = [0]

            def tywait():
                vector.wait_ge(s_ty, tyc[0])

            def tyinc(inst):
                tyc[0] += 1
                inst.then_inc(s_ty, 1)

            for c in range(NC):
                # chain tiny-ops for o5 = c-6 are interleaved between big ops
                # so each s_ty write-ack returns during a big op (no stall).
                # s_ty layout: 5 ops per o: [mu/E2, negvar, recip, s, tneg]
                o5 = c - 6
                ch = 1 <= o5 <= NO
                m = c - 4
                mx = 1 <= m <= NO
                o = c - 7
                p2 = 1 <= o <= NO
                mq = c - 1
                if 0 <= mq <= S - 1:
                    # q(mq) = gx2 + gy2
                    vector.wait_ge(s_sq, 4 * (mq + 1))
                    if mq >= 2:
                        vector.wait_ge(s_sqrt, mq - 1)  # qsl slot free
                    qv = qsl_t[mq % 2][:, :].rearrange("p (qq x) -> p qq x",
                                                       x=QC)
                    tt(qv, gxy2_half(mq, 0), gxy2_half(mq, QC),
                       ALU.add).then_inc(s_q, 1)
                mf = c - 2
                if 0 <= mf <= S - 1:
                    # feat(mf) = gms + lapa
                    vector.wait_ge(s_sqrt, mf + 1)
                    vector.wait_ge(s_psf, 4 * (mf + 1))
                    tt(feat_t[mf % FR][:, :], gms_t[mf % 2][:, :],
                       lapa_t[mf % LR][:, :], ALU.add).then_inc(s_feat, 1)
                oA = c - 7
                if 1 <= oA <= NO:
                    # A4(oA): copy bcast psum -> sbuf (was on ACT)
                    vector.wait_ge(s_bc, oA)
                    vector.tensor_copy(
                        sb_st[oA % 2][:, :],
                        psum_bc[:, (oA % 2) * 2:(oA % 2) * 2 + 2]) \
                        .then_inc(s_stsb, 1)
                if ch:
                    # chain_a.1: [mu, E2] = psum_stat / NTOT
                    vector.wait_ge(s_statmm, o5)
                    base = (o5 % 2) * 2
                    tyinc(ts(sc_mu[0:1, :], psum_stat[0:1, base:base + 2],
                             1.0 / NTOT, None, ALU.mult))
                if mx:
                    oc = m - 1
                    vector.wait_ge(s_feat, m + 1)
                    ts(t1_t[:, :], feat_t[(m - 1) % FR][:, :],
                       wmix_t[:, oc:oc + 1], None, ALU.mult)
                if ch:
                    # chain_a.2: negvar = mu^2 - E2
                    if o5 >= 3:
                        vector.wait_ge(s_std, o5 - 2)
                    tywait()
                    tyinc(stt(sc_var[o5 % 2][0:1, :], sc_mu[0:1, 0:1],
                              sc_mu[0:1, 0:1], sc_mu[0:1, 1:2],
                              ALU.mult, ALU.subtract))
                if mx:
                    ts(t2_t[:, :], feat_t[m % FR][:, :],
                       wmix_t[:, NO + oc:NO + oc + 1], None, ALU.mult)
                    tt(a_t[:, :], t1_t[:, :], t2_t[:, :], ALU.add)
                if p2:
                    ts(inner_t[:, :], xb_int(o), one_mg,
                       sb_st[o % 2][:, 1:2], ALU.mult, ALU.subtract)
                if ch:
                    # chain_b.1: inv = 1/std   (A3 ran on ACT q2 last slot)
                    vector.wait_ge(s_std, o5)
                    tyinc(vector.reciprocal(sc_inv[0:1, :], sc_std[o5 % 2][0:1, :]))
                if mx:
                    vector.wait_ge(s_feat, m + 2)
                    if m >= 3:
                        vector.wait_ge(s_statmm, m - 2)  # stats slot free
                    stt(mixed_t[m % MR][:, :], feat_t[(m + 1) % FR][:, :],
                        wmix_t[:, 2 * NO + oc:2 * NO + oc + 1], a_t[:, :],
                        ALU.mult, ALU.add,
                        accum_out=stats_t[m % 2][:, 0:1]).then_inc(s_mixd, 1)
                if ch:
                    # chain_b.2: s = inv * (gamma bn_w)
                    occ = o5 - 1
                    if o5 >= 3:
                        vector.wait_ge(s_bc, o5 - 2)
                    tywait()
                    tyinc(tt(st_vec[o5 % 2][0:1, 0:1], sc_inv[0:1, :],
                             wbn_t[0:1, occ:occ + 1], ALU.mult))
                if ch:
                    # chain_b.3: tneg = s*mu - (gamma bn_b)
                    tywait()
                    tyinc(stt(st_vec[o5 % 2][0:1, 1:2], sc_mu[0:1, 0:1],
                              st_vec[o5 % 2][0:1, 0:1],
                              wbn_t[0:1, NO + occ:NO + occ + 1],
                              ALU.mult, ALU.subtract))
                if p2:
                    ts(scr2_t[:, :], mixed_t[o % MR][:, :],
                       sb_st[o % 2][:, 0:1], None, ALU.mult)
                    if o >= 3:
                        vector.wait_ge(s_do[o % 2], 16 * ((o - 1) // 2))
                    tt(out_sb[o % 2][:, :], scr2_t[:, :], inner_t[:, :],
                       ALU.add).then_inc(s_v2, 1)

    ctx.close()
    return nc


def make_scalars(alpha_gm, alpha_lap, gamma_p):
    a_gm = float(_softplus(np.float64(alpha_gm)))
    a_lap = float(_softplus(np.float64(alpha_lap)))
    gamma = float(1.0 / (1.0 + math.exp(-float(gamma_p))))
    return {"a_gm": a_gm, "a_lap": a_lap, "gamma": gamma}


def host_prepare(x, P, alpha_gm, alpha_lap, gamma_p, bn_weight, bn_bias,
                 n_cores=8):
    Bt, C, Hh, Ww = x.shape
    NO = C // n_cores
    S = NO + 2
    scal = make_scalars(alpha_gm, alpha_lap, gamma_p)
    gamma = scal["gamma"]

    sp = _softplus(P.astype(np.float64)).astype(np.float32)
    w_di = np.diag(sp).copy() + MIX_EPS
    w_lo = np.zeros(C, np.float32)
    w_hi = np.zeros(C, np.float32)
    w_lo[1:] = sp[np.arange(1, C), np.arange(0, C - 1)]
    w_hi[:-1] = sp[np.arange(0, C - 1), np.arange(1, C)]

    xb_all = np.zeros((C + 2, Hh, Bt, WP), dtype=np.float16)
    xb_all[1:C + 1, :, :, 2:2 + Ww] = np.ascontiguousarray(
        x.transpose(1, 2, 0, 3)).astype(np.float16)

    cw = np.ascontiguousarray(conv_stationaries())

    in_maps = []
    for r in range(n_cores):
        lo = r * NO
        sl = slice(lo, lo + NO)
        wmix = np.zeros((Hh, 3 * NO), np.float32)
        wmix[:, 0:NO] = w_lo[sl]
        wmix[:, NO:2 * NO] = w_di[sl]
        wmix[:, 2 * NO:3 * NO] = w_hi[sl]
        wbn = np.zeros((1, 2 * NO), np.float32)
        wbn[0, 0:NO] = gamma * bn_weight[sl]
        wbn[0, NO:2 * NO] = gamma * bn_bias[sl]
        in_maps.append({
            "xb": np.ascontiguousarray(xb_all[lo:lo + S]),
            "cw": cw,
            "wmix": wmix,
            "wbn": wbn,
            "onesv": np.ones((1, H), np.float32),
            "cb": np.repeat(np.array([[scal["a_gm"] ** 2 * 1e-6, BN_EPS, 1.0]],
                                     np.float32), H, axis=0),
        })
    return in_maps, scal, {"S": S, "NO": NO, "B": Bt}


def assemble_out(results, NO, n_cores=8):
    outs = []
    for r in range(n_cores):
        o = results[r]["out"]            # [NO, H, B, W] fp16
        outs.append(o.astype(np.float32).transpose(2, 0, 1, 3))
    return np.concatenate(outs, axis=1)


# ---------------------------------------------------------------------------
# Self-contained entry point: kernel(**inputs) -> np.ndarray
# ---------------------------------------------------------------------------
import types as _types


def _install_axon_profile_shim():
    """Make run_bass_kernel_spmd usable in this container (no antenv hooks)."""
    import sys as _sys
    try:
        from antenv import axon_hooks  # noqa: F401
        return
    except ImportError:
        pass
    try:
        from trn_agent_boot.trn_boot import _ntff_profile_via_ctypes
        mod = _types.ModuleType('antenv.axon_hooks')
        _hook = _ntff_profile_via_ctypes('/opt/axon/libaxon_pjrt.so')
        mod.get_axon_ntff_profile_hook = lambda: _hook
        mod.set_axon_ntff_profile_hook = lambda h: None
        _sys.modules['antenv.axon_hooks'] = mod
        import antenv
        antenv.axon_hooks = mod
        from concourse import bass_utils
        bass_utils.upload_artifacts = lambda tmpdir: f"local://{tmpdir}"
    except Exception:
        pass


_NC_CACHE = {}


def kernel(**inputs):
    from concourse.bass_utils import run_bass_kernel_spmd
    _install_axon_profile_shim()
    x = np.asarray(inputs["x"], dtype=np.float32)
    P = np.asarray(inputs["P"], dtype=np.float32)
    a_gm = float(np.asarray(inputs["alpha_gm"]))
    a_lap = float(np.asarray(inputs["alpha_lap"]))
    g_p = float(np.asarray(inputs["gamma_p"]))
    bn_w = np.asarray(inputs["bn_weight"], dtype=np.float32)
    bn_b = np.asarray(inputs["bn_bias"], dtype=np.float32)

    n_cores = 8
    B, C = x.shape[0], x.shape[1]
    NO = C // n_cores
    NQ = 4 if (B % 4 == 0) else (2 if B % 2 == 0 else 1)

    in_maps, scal, meta = host_prepare(x, P, a_gm, a_lap, g_p, bn_w, bn_b,
                                       n_cores=n_cores)
    key = (meta["S"], B, NQ, round(scal["a_gm"], 9), round(scal["a_lap"], 9),
           round(scal["gamma"], 9))
    nc = _NC_CACHE.get(key)
    if nc is None:
        nc = build_nc(meta["S"], B, NQ, scal, n_cores=n_cores)
        _NC_CACHE[key] = nc
    res = run_bass_kernel_spmd(nc, in_maps, core_ids=list(range(n_cores)),
                               trace=False)
    out = assemble_out(res.results, NO, n_cores)
    return out.astype(np.float32)


# revision 25
# speedup vs baseline: 1.0016x; 1.0016x over previous
import sys as _sys
for _p in ('/opt/trn_rl_repo',):
    if _p not in _sys.path:
        _sys.path.insert(0, _p)
"""AnalyticBlock Trainium kernel: channel-sharded SPMD across 8 cores.

Per core: S = 34 channel slabs (1 halo each side), B batch, 128x128
images. Layout per slab: SBUF [h=128 partitions, (b, w_pad=132)], fp16.

Software-pipelined slot schedule (slot c, PE quarters q0..q3):
  PE  : 8 banded matmuls/quarter -> gx,gy,lap in PSUM (2D conv via
        vertical stationaries x horizontally shifted moving APs);
        T3(c-7) s,t broadcast @q1; T2(c-5) stats partition-reduce @q3
  ACT : sumsq(c-5) = Square(mixed)+accum_out at slot start (fills the
        PE-wait boundary gap); per quarter Square (psum->gxy2 fp16,
        quarter-contiguous) + Abs (psum lap -> lapa); A3(c-6) std
        sqrt @q1; full-slab Sqrt(c-1) @q2
  DVE : q(c-1) = gx2+gy2 (tt); feat(c-2) = gms+lapa (tt); A4(c-7)
        bcast copy; mix(c-4): t1,t2 (ts 4x), a (tt 2x), mixed (stt
        + sum accum_out); pass2(c-7): inner (ts), scr2 (ts),
        out = scr2+inner (tt, fp16 out); BN scalar chains (c-6)
        interleaved between big ops so s_ty write-acks never stall
  Chains: negvar = mu^2-E2 (stt); std = Sqrt(-negvar+eps) on ACT;
        s = 1/std * gamma*bn_w; tneg = s*mu - gamma*bn_b;
        inner = (1-gamma)*x - tneg.
  BN is per-channel over (B,H,W): fully local under channel sharding,
  no collectives. Output written fp16 (halves DMA), upcast on host.
"""
import math
import numpy as np
import ml_dtypes

import concourse.bass as bass
import concourse.mybir as mybir

F32 = mybir.dt.float32
FP16 = mybir.dt.float16
ALU = mybir.AluOpType
ACTF = mybir.ActivationFunctionType

BN_EPS = 1e-5
MIX_EPS = 1e-3
H = 128
W = 128
WP = 132  # 2 zero pad cols each side


def _softplus(x):
    return np.logaddexp(0.0, x)


def conv_stationaries():
    """6 stationaries [K=128, M=128], lhsT layout: lhsT[k, m] = M[m, k]."""
    n = H
    i = np.arange(n)
    Vs = np.zeros((n, n), np.float32)   # vertical smooth [1,2,1]
    Vs[i, i] = 2.0
    Vs[i[1:], i[1:] - 1] = 1.0
    Vs[i[:-1], i[:-1] + 1] = 1.0
    Vd = np.zeros((n, n), np.float32)   # vertical corr [1,0,-1]
    Vd[i[1:], i[1:] - 1] = 1.0
    Vd[i[:-1], i[:-1] + 1] = -1.0
    VL = np.zeros((n, n), np.float32)   # vertical lap incl center
    VL[i, i] = -4.0
    VL[i[1:], i[1:] - 1] = 1.0
    VL[i[:-1], i[:-1] + 1] = 1.0
    I = np.eye(n, dtype=np.float32)
    mats = [Vs, -Vs, Vd, 2.0 * Vd, VL, I]
    cw = np.stack([m.T for m in mats], axis=1)  # [K=128, 6, M=128]
    return cw.astype(np.float16)


def build_nc(S, B, NQ, scal, n_cores=8, debug=False):
    NO = S - 2
    BQ = B // NQ
    QC = BQ * W            # cols per quarter
    COLS = B * W           # cols per slab
    NTOT = B * H * W
    a2 = scal["a_gm"] ** 2
    a_lap = scal["a_lap"]
    one_mg = 1.0 - scal["gamma"]

    XR, FR, MR, LR = 11, 4, 4, 3
    NC = S + 7             # slots: DMA(o=c-8) runs to c = NO+8 = S+6

    nc = bass.Bass(target_bir_lowering=False, detect_race_conditions=False)

    xb_ext = nc.declare_dram_parameter("xb", [S, H, B, WP], FP16, isOutput=False)
    cw_ext = nc.declare_dram_parameter("cw", [H, 6, H], FP16, isOutput=False)
    wmix_ext = nc.declare_dram_parameter("wmix", [H, 3 * NO], F32, isOutput=False)
    wbn_ext = nc.declare_dram_parameter("wbn", [1, 2 * NO], F32, isOutput=False)
    ones_ext = nc.declare_dram_parameter("onesv", [1, H], F32, isOutput=False)
    cb_ext = nc.declare_dram_parameter("cb", [H, 3], F32, isOutput=False)
    out_ext = nc.declare_dram_parameter("out", [NO, H, B, W], FP16, isOutput=True)

    from contextlib import ExitStack
    ctx = ExitStack()
    sb = lambda name, shape, dt: ctx.enter_context(nc.sbuf_tensor(name, shape, dt))
    ps = lambda name, shape: ctx.enter_context(nc.psum_tensor(name, shape, F32))
    sem = lambda name: ctx.enter_context(nc.semaphore(name))

    xb_t = [sb(f"xb{j}", [H, B * WP], FP16) for j in range(XR)]
    cw_t = sb("cw_s", [H, 6 * H], FP16)
    wmix_t = sb("wmix_s", [H, 3 * NO], F32)
    wbn_t = sb("wbn_s", [1, 2 * NO], F32)
    ones_bc = sb("ones_bc", [1, H], F32)
    cb_t = sb("cb_s", [H, 3], F32)   # col0 sqrt bias, col1 BN_EPS, col2 1.0

    feat_t = [sb(f"feat{j}", [H, COLS], FP16) for j in range(FR)]
    mixed_t = [sb(f"mixed{j}", [H, COLS], FP16) for j in range(MR)]
    gxy2_t = [sb(f"gxy2_{j}", [H, 2 * COLS], FP16) for j in range(2)]
    lapa_t = [sb(f"lapa{j}", [H, COLS], FP16) for j in range(LR)]
    qsl_t = [sb(f"qsl{j}", [H, COLS], FP16) for j in range(2)]
    gms_t = [sb(f"gms{j}", [H, COLS], FP16) for j in range(2)]
    t1_t = sb("t1", [H, COLS], FP16)
    t2_t = sb("t2", [H, COLS], FP16)
    a_t = sb("a_t", [H, COLS], FP16)
    gscr = sb("gscr", [H, COLS], FP16)
    inner_t = sb("inner", [H, COLS], FP16)
    scr2_t = sb("scr2", [H, COLS], FP16)
    out_sb = [sb(f"outsb{j}", [H, COLS], FP16) for j in range(2)]
    stats_t = [sb(f"stats{j}", [H, 2], F32) for j in range(2)]
    sc_mu = sb("sc_mu", [1, 2], F32)
    sc_tmp = sb("sc_tmp", [1, 1], F32)
    sc_var = [sb(f"sc_var{j}", [1, 1], F32) for j in range(2)]
    sc_std = [sb(f"sc_std{j}", [1, 1], F32) for j in range(2)]
    sc_inv = sb("sc_inv", [1, 1], F32)
    st_vec = [sb(f"st_vec{j}", [1, 2], F32) for j in range(2)]
    sb_st = [sb(f"sb_st{j}", [H, 2], F32) for j in range(2)]

    psum_gxy = [ps(f"psgxy{j}", [H, 2 * QC]) for j in range(2)]
    psum_lap = [ps(f"pslap{j}", [H, QC]) for j in range(2)]
    psum_stat = ps("psstat", [1, 4])
    psum_bc = ps("psbc", [H, 4])

    s_cst = sem("s_cst")
    s_xb = [sem(f"s_xb{j}") for j in range(XR)]
    s_mm = sem("s_mm")        # PE conv quarter done (count 4c+q+1)
    s_sq = sem("s_sq")        # ACT Square done (count 4c+q+1)
    s_psf = sem("s_psf")      # ACT Abs done -> psum slot free (count 4c+q+1)
    s_q = sem("s_q")          # GPS qadd(m) done (count m+1)
    s_sqrt = sem("s_sqrt")    # ACT Sqrt(m) done (count m+1)
    s_feat = sem("s_feat")    # GPS feat(m) done (count m+1)
    s_mixd = sem("s_mixd")    # DVE mixed(m) done (count m)
    s_ssq = sem("s_ssq")      # DVE sumsq(m) done (count m)
    s_statmm = sem("s_statmm")  # PE T2(o) done (count o)
    s_std = sem("s_std")      # ACT A3(o) done (count o)
    s_bc = sem("s_bc")        # PE T3(o) done (count o)
    s_stsb = sem("s_stsb")    # ACT A4(o) done (count o)
    s_v2 = sem("s_v2")        # DVE out(o) done (count o)
    s_ty = sem("s_ty")        # DVE tiny-op write-ack ordering
    s_do = [sem(f"s_do{j}") for j in range(2)]

    def xb_q(c, q, sh):
        v = xb_t[c % XR][:, :].rearrange("p (b w) -> p b w", w=WP)
        return v[:, q * BQ:(q + 1) * BQ, sh:sh + W]

    def xb_int(c):
        v = xb_t[c % XR][:, :].rearrange("p (b w) -> p b w", w=WP)
        return v[:, :, 2:2 + W]

    def cwm(j):
        return cw_t[:, j * H:(j + 1) * H]

    def gxy2_view(c, q):
        # quarter-contiguous [gx|gy] block: no strided ACT writes
        return gxy2_t[c % 2][:, q * 2 * QC:(q + 1) * 2 * QC]

    def gxy2_half(c, off):
        v = gxy2_t[c % 2][:, :].rearrange("p (qq x) -> p qq x", x=2 * QC)
        return v[:, :, off:off + QC]

    with nc.Block() as block:

        @block.sync
        def _(sync):
            sync.dma_start(out=cw_t[:, :], in_=cw_ext[:, :, :]).then_inc(s_cst, 16)
            sync.dma_start(out=wmix_t[:, :], in_=wmix_ext[:, :]).then_inc(s_cst, 16)
            sync.dma_start(out=wbn_t[:, :], in_=wbn_ext[:, :]).then_inc(s_cst, 16)
            sync.dma_start(out=ones_bc[0:1, :], in_=ones_ext[0:1, :]).then_inc(s_cst, 16)
            sync.dma_start(out=cb_t[:, :], in_=cb_ext[:, :]).then_inc(s_cst, 16)
            for c0 in range(min(3, S)):
                sync.dma_start(out=xb_t[c0 % XR][:, :],
                               in_=xb_ext[c0, :, :, :]).then_inc(s_xb[c0 % XR], 16)
            for c in range(NC):
                cl = c + 3
                if cl < S:
                    j = cl % XR
                    if cl >= XR:
                        sync.wait_ge(s_v2, cl - XR)
                    sync.dma_start(out=xb_t[j][:, :],
                                   in_=xb_ext[cl, :, :, :]).then_inc(s_xb[j], 16)
                o = c - 8
                if 1 <= o <= NO:
                    sync.wait_ge(s_v2, o)
                    sync.dma_start(out=out_ext[o - 1, :, :, :],
                                   in_=out_sb[o % 2][:, :]).then_inc(s_do[o % 2], 16)

        @block.tensor
        def _(tensor):
            tensor.wait_ge(s_cst, 80)
            for c in range(NC):
                for q in range(NQ):
                    if q == 1:
                        # T3(o = c-7): broadcast s,t -> [H,2] psum
                        o3 = c - 7
                        if 1 <= o3 <= NO:
                            tensor.wait_ge(s_ty, 5 * o3)
                            if o3 >= 3:
                                tensor.wait_ge(s_stsb, o3 - 2)
                            tensor.matmul(
                                psum_bc[:, (o3 % 2) * 2:(o3 % 2) * 2 + 2],
                                ones_bc[0:1, :], st_vec[o3 % 2][0:1, :],
                                start=True, stop=True).then_inc(s_bc, 1)
                    if c <= S - 1:
                        it = c * NQ + q
                        if q == 0:
                            tensor.wait_ge(s_xb[c % XR], (c // XR + 1) * 16)
                        if it >= 2:
                            tensor.wait_ge(s_psf, it - 1)
                        slot = it % 2
                        g = psum_gxy[slot]
                        l = psum_lap[slot]
                        mm = tensor.matmul
                        mm(g[:, 0:QC], cwm(0), xb_q(c, q, 1), start=True, stop=False)
                        mm(g[:, 0:QC], cwm(1), xb_q(c, q, 3), start=False, stop=True)
                        mm(g[:, QC:2 * QC], cwm(2), xb_q(c, q, 1), start=True, stop=False)
                        mm(g[:, QC:2 * QC], cwm(2), xb_q(c, q, 3), start=False, stop=False)
                        mm(g[:, QC:2 * QC], cwm(3), xb_q(c, q, 2), start=False, stop=True)
                        mm(l[:, :], cwm(5), xb_q(c, q, 1), start=True, stop=False)
                        mm(l[:, :], cwm(5), xb_q(c, q, 3), start=False, stop=False)
                        mm(l[:, :], cwm(4), xb_q(c, q, 2), start=False, stop=True) \
                            .then_inc(s_mm, 1)
                # T2(o = c-4): stats partition-reduce
                o2 = c - 5
                if 1 <= o2 <= NO:
                    tensor.wait_ge(s_ssq, o2)
                    if o2 >= 3:
                        tensor.wait_ge(s_ty, 5 * (o2 - 3) + 1)
                    tensor.matmul(psum_stat[0:1, (o2 % 2) * 2:(o2 % 2) * 2 + 2],
                                  cb_t[:, 2:3], stats_t[o2 % 2][:, :],
                                  start=True, stop=True).then_inc(s_statmm, 1)

        @block.scalar
        def _(scalar):
            act = scalar.activation
            scalar.wait_ge(s_cst, 80)
            for c in range(NC):
                # sumsq(m2 = c-5): PE-independent; fills slot-boundary gap
                m2 = c - 5
                if 1 <= m2 <= NO:
                    scalar.wait_ge(s_mixd, m2)
                    if m2 >= 3:
                        scalar.wait_ge(s_statmm, m2 - 2)
                    act(gscr[:, :], mixed_t[m2 % MR][:, :],
                        ACTF.Square,
                        accum_out=stats_t[m2 % 2][:, 1:2]).then_inc(s_ssq, 1)
                for q in range(NQ):
                    if c <= S - 1:
                        it = c * NQ + q
                        slot = it % 2
                        scalar.wait_ge(s_mm, it + 1)
                        if q == 0 and c >= 2:
                            scalar.wait_ge(s_q, c - 1)      # gxy2 slot free
                        act(gxy2_view(c, q), psum_gxy[slot][:, :],
                            ACTF.Square).then_inc(s_sq, 1)
                        if q == 0 and c >= 3:
                            scalar.wait_ge(s_feat, c - 2)   # lapa slot free
                        act(lapa_t[c % LR][:, q * QC:(q + 1) * QC],
                            psum_lap[slot][:, :], ACTF.Abs,
                            scale=a_lap).then_inc(s_psf, 1)
                    if q == 1:
                        # A3(o = c-6): std = sqrt(-negvar + eps)
                        o = c - 6
                        if 1 <= o <= NO:
                            scalar.wait_ge(s_ty, 5 * (o - 1) + 2)
                            if o >= 3:
                                scalar.wait_ge(s_ty, 5 * (o - 3) + 3)
                            act(sc_std[o % 2][0:1, :], sc_var[o % 2][0:1, :],
                                ACTF.Sqrt, bias=cb_t[0:1, 1:2],
                                scale=-1.0).then_inc(s_std, 1)
                    if q == 2:
                        # full-slab Sqrt(m = c-1)
                        m = c - 1
                        if 0 <= m <= S - 1:
                            scalar.wait_ge(s_q, m + 1)
                            if m >= 2:
                                scalar.wait_ge(s_feat, m - 1)  # gms slot free
                            act(gms_t[m % 2][:, :], qsl_t[m % 2][:, :],
                                ACTF.Sqrt, bias=cb_t[:, 0:1],
                                scale=a2).then_inc(s_sqrt, 1)

        @block.vector
        def _(vector):
            vector.wait_ge(s_cst, 80)
            stt = vector.scalar_tensor_tensor
            ts = vector.tensor_scalar
            tt = vector.tensor_tensor

            tyc = [0]

            def tywait():
                vector.wait_ge(s_ty, tyc[0])

            def tyinc(inst):
                tyc[0] += 1
                inst.then_inc(s_ty, 1)

            for c in range(NC):
                # chain tiny-ops for o5 = c-6 are interleaved between big ops
                # so each s_ty write-ack returns during a big op (no stall).
                # s_ty layout: 5 ops per o: [mu/E2, negvar, recip, s, tneg]
                o5 = c - 6
                ch = 1 <= o5 <= NO
                m = c - 4
                mx = 1 <= m <= NO
                o = c - 7
                p2 = 1 <= o <= NO
                mq = c - 1
                if 0 <= mq <= S - 1:
                    # q(mq) = gx2 + gy2
                    vector.wait_ge(s_sq, 4 * (mq + 1))
                    if mq >= 2:
                        vector.wait_ge(s_sqrt, mq - 1)  # qsl slot free
                    qv = qsl_t[mq % 2][:, :].rearrange("p (qq x) -> p qq x",
                                                       x=QC)
                    tt(qv, gxy2_half(mq, 0), gxy2_half(mq, QC),
                       ALU.add).then_inc(s_q, 1)
                mf = c - 2
                if 0 <= mf <= S - 1:
                    # feat(mf) = gms + lapa
                    vector.wait_ge(s_sqrt, mf + 1)
                    vector.wait_ge(s_psf, 4 * (mf + 1))
                    tt(feat_t[mf % FR][:, :], gms_t[mf % 2][:, :],
                       lapa_t[mf % LR][:, :], ALU.add).then_inc(s_feat, 1)
                oA = c - 7
                if 1 <= oA <= NO:
                    # A4(oA): copy bcast psum -> sbuf (was on ACT)
                    vector.wait_ge(s_bc, oA)
                    vector.tensor_copy(
                        sb_st[oA % 2][:, :],
                        psum_bc[:, (oA % 2) * 2:(oA % 2) * 2 + 2]) \
                        .then_inc(s_stsb, 1)
                if ch:
                    # chain_a.1: [mu, E2] = psum_stat / NTOT
                    vector.wait_ge(s_statmm, o5)
                    base = (o5 % 2) * 2
                    tyinc(ts(sc_mu[0:1, :], psum_stat[0:1, base:base + 2],
                             1.0 / NTOT, None, ALU.mult))
                if mx:
                    oc = m - 1
                    vector.wait_ge(s_feat, m + 1)
                    ts(t1_t[:, :], feat_t[(m - 1) % FR][:, :],
                       wmix_t[:, oc:oc + 1], None, ALU.mult)
                if ch:
                    # chain_a.2: negvar = mu^2 - E2
                    if o5 >= 3:
                        vector.wait_ge(s_std, o5 - 2)
                    tywait()
                    tyinc(stt(sc_var[o5 % 2][0:1, :], sc_mu[0:1, 0:1],
                              sc_mu[0:1, 0:1], sc_mu[0:1, 1:2],
                              ALU.mult, ALU.subtract))
                if mx:
                    ts(t2_t[:, :], feat_t[m % FR][:, :],
                       wmix_t[:, NO + oc:NO + oc + 1], None, ALU.mult)
                    tt(a_t[:, :], t1_t[:, :], t2_t[:, :], ALU.add)
                if p2:
                    ts(inner_t[:, :], xb_int(o), one_mg,
                       sb_st[o % 2][:, 1:2], ALU.mult, ALU.subtract)
                if ch:
                    # chain_b.1: inv = 1/std   (A3 ran on ACT q2 last slot)
                    vector.wait_ge(s_std, o5)
                    tyinc(vector.reciprocal(sc_inv[0:1, :], sc_std[o5 % 2][0:1, :]))
                if mx:
                    vector.wait_ge(s_feat, m + 2)
                    if m >= 3:
                        vector.wait_ge(s_statmm, m - 2)  # stats slot free
                    stt(mixed_t[m % MR][:, :], feat_t[(m + 1) % FR][:, :],
                        wmix_t[:, 2 * NO + oc:2 * NO + oc + 1], a_t[:, :],
                        ALU.mult, ALU.add,
                        accum_out=stats_t[m % 2][:, 0:1]).then_inc(s_mixd, 1)
                if ch:
                    # chain_b.2: s = inv * (gamma bn_w)
                    occ = o5 - 1
                    if o5 >= 3:
                        vector.wait_ge(s_bc, o5 - 2)
                    tywait()
                    tyinc(tt(st_vec[o5 % 2][0:1, 0:1], sc_inv[0:1, :],
                             wbn_t[0:1, occ:occ + 1], ALU.mult))
                if ch:
                    # chain_b.3: tneg = s*mu - (gamma bn_b)
                    tywait()
                    tyinc(stt(st_vec[o5 % 2][0:1, 1:2], sc_mu[0:1, 0:1],
                              st_vec[o5 % 2][0:1, 0:1],
                              wbn_t[0:1, NO + occ:NO + occ + 1],
                              ALU.mult, ALU.subtract))
                if p2:
                    ts(scr2_t[:, :], mixed_t[o % MR][:, :],
                       sb_st[o % 2][:, 0:1], None, ALU.mult)
                    if o >= 3:
                        vector.wait_ge(s_do[o % 2], 16 * ((o - 1) // 2))
                    tt(out_sb[o % 2][:, :], scr2_t[:, :], inner_t[:, :],
                       ALU.add).then_inc(s_v2, 1)

    ctx.close()
    return nc


def make_scalars(alpha_gm, alpha_lap, gamma_p):
    a_gm = float(_softplus(np.float64(alpha_gm)))
    a_lap = float(_softplus(np.float64(alpha_lap)))
    gamma = float(1.0 / (1.0 + math.exp(-float(gamma_p))))
    return {"a_gm": a_gm, "a_lap": a_lap, "gamma": gamma}


def host_prepare(x, P, alpha_gm, alpha_lap, gamma_p, bn_weight, bn_bias,
                 n_cores=8):
    Bt, C, Hh, Ww = x.shape
    NO = C // n_cores
    S = NO + 2
    scal = make_scalars(alpha_gm, alpha_lap, gamma_p)
    gamma = scal["gamma"]

    sp = _softplus(P.astype(np.float64)).astype(np.float32)
    w_di = np.diag(sp).copy() + MIX_EPS
    w_lo = np.zeros(C, np.float32)
    w_hi = np.zeros(C, np.float32)
    w_lo[1:] = sp[np.arange(1, C), np.arange(0, C - 1)]
    w_hi[:-1] = sp[np.arange(0, C - 1), np.arange(1, C)]

    xb_all = np.zeros((C + 2, Hh, Bt, WP), dtype=np.float16)
    xb_all[1:C + 1, :, :, 2:2 + Ww] = np.ascontiguousarray(
        x.transpose(1, 2, 0, 3)).astype(np.float16)

    cw = np.ascontiguousarray(conv_stationaries())

    in_maps = []
    for r in range(n_cores):
        lo = r * NO
        sl = slice(lo, lo + NO)
        wmix = np.zeros((Hh, 3 * NO), np.float32)
        wmix[:, 0:NO] = w_lo[sl]
        wmix[:, NO:2 * NO] = w_di[sl]
        wmix[:, 2 * NO:3 * NO] = w_hi[sl]
        wbn = np.zeros((1, 2 * NO), np.float32)
        wbn[0, 0:NO] = gamma * bn_weight[sl]
        wbn[0, NO:2 * NO] = gamma * bn_bias[sl]
        in_maps.append({
            "xb": np.ascontiguousarray(xb_all[lo:lo + S]),
            "cw": cw,
            "wmix": wmix,
            "wbn": wbn,
            "onesv": np.ones((1, H), np.float32),
            "cb": np.repeat(np.array([[scal["a_gm"] ** 2 * 1e-6, BN_EPS, 1.0]],
                                     np.float32), H, axis=0),
        })
    return in_maps, scal, {"S": S, "NO": NO, "B": Bt}


def assemble_out(results, NO, n_cores=8):
    outs = []
    for r in range(n_cores):
        o = results[r]["out"]            # [NO, H, B, W] fp16
        outs.append(o.astype(np.float32).transpose(2, 0, 1, 3))
    return np.concatenate(outs, axis=1)


# ---------------------------------------------------------------------------
# Self-contained entry point: kernel(**inputs) -> np.ndarray
# ---------------------------------------------------------------------------
import types as _types


def _install_axon_profile_shim():
    """Make run_bass_kernel_spmd usable in this container (no antenv hooks)."""
    import sys as _sys
    try:
        from antenv import axon_hooks  # noqa: F401
        return
    except ImportError:
        pass
    try:
        from trn_agent_boot.trn_boot import _ntff_profile_via_ctypes
        mod = _types.ModuleType('antenv.axon_hooks')
        _hook = _ntff_profile_via_ctypes('/opt/axon/libaxon_pjrt.so')
        mod.get_axon_ntff_profile_hook = lambda: _hook
        mod.set_axon_ntff_profile_hook = lambda h: None
        _sys.modules['antenv.axon_hooks'] = mod
        import antenv
        antenv.axon_hooks = mod
        from concourse import bass_utils
        bass_utils.upload_artifacts = lambda tmpdir: f"local://{tmpdir}"
    except Exception:
        pass


_NC_CACHE = {}


def kernel(**inputs):
    from concourse.bass_utils import run_bass_kernel_spmd
    _install_axon_profile_shim()
    x = np.asarray(inputs["x"], dtype=np.float32)
    P = np.asarray(inputs["P"], dtype=np.float32)
    a_gm = float(np.asarray(inputs["alpha_gm"]))
    a_lap = float(np.asarray(inputs["alpha_lap"]))
    g_p = float(np.asarray(inputs["gamma_p"]))
    bn_w = np.asarray(inputs["bn_weight"], dtype=np.float32)
    bn_b = np.asarray(inputs["bn_bias"], dtype=np.float32)

    n_cores = 8
    B, C = x.shape[0], x.shape[1]
    NO = C // n_cores
    NQ = 4 if (B % 4 == 0) else (2 if B % 2 == 0 else 1)

    in_maps, scal, meta = host_prepare(x, P, a_gm, a_lap, g_p, bn_w, bn_b,
                                       n_cores=n_cores)
    key = (meta["S"], B, NQ, round(scal["a_gm"], 9), round(scal["a_lap"], 9),
           round(scal["gamma"], 9))
    nc = _NC_CACHE.get(key)
    if nc is None:
        nc = build_nc(meta["S"], B, NQ, scal, n_cores=n_cores)
        _NC_CACHE[key] = nc
    res = run_bass_kernel_spmd(nc, in_maps, core_ids=list(range(n_cores)),
                               trace=False)
    out = assemble_out(res.results, NO, n_cores)
    return out.astype(np.float32)
